# revision 10
# baseline (speedup 1.0000x reference)
"""Day-routed adapter MLP (per-sample day-specific 2-layer MLP + LayerNorm)
for 8 Trainium2 NeuronCores.

Computation per sample b (day d = day_indices[b]):
    h = relu(x[b] @ W1[d] + b1[d])        # [T, D_hid]
    y = h @ W2[d] + b2[d]                 # [T, D_out]
    out = LN(y) * gamma[d] + beta[d]      # LN over last dim

Sharding: data-parallel over batch, 8 samples per core. The per-sample
day weights are gathered on the host (routing is host-visible), and x is
pre-transposed on the host so the device needs no transposes at all:

  pass 1:  hT[h_chunk, :T] += W1[k_chunk, h_chunk].T @ xT[k_chunk, :T]
           (lhsT = W1 natural layout, rhs = xT)  -> hT with H on partitions,
           so b1 is a per-partition bias fused into the ReLU copyback (ACT).
  pass 2:  y[t_tile, :O]  += hT[k_chunk, t_tile].T @ W2[k_chunk, :O]
           (lhsT = hT from pass 1, rhs = W2 natural layout) -> y with T on
           partitions and O on the free axis, which is exactly the layout
           LayerNorm wants (bn_stats/bn_aggr reduce along free axis).
"""

import os

import numpy as np
import ml_dtypes

import concourse.bass as bass
import concourse.mybir as mybir
import concourse.tile as tile
from concourse import bacc
from concourse.bass_utils import run_bass_kernel_spmd

N_CORES = 8
B, T, D_IN = 64, 512, 512
D_HID, D_OUT = 1024, 512
S = B // N_CORES  # samples per core
EPS = 1e-5

P = 128
KD = D_IN // P   # 4 contraction chunks in pass 1
KH = D_HID // P  # 8 contraction chunks in pass 2 (= H chunks of pass 1 out)
MT = T // P      # 4 token tiles in pass 2

# Matmul input dtype. float16: full PE rate (1 cyc/row, FWL hides weight
# loads), half the DMA bytes of fp32, and a 10-bit mantissa (~4x better than
# bf16; fp32 accumulate in PSUM). float32r: fp32 storage but ~2 cyc/row and
# 2x the DMA traffic. bfloat16: same speed as fp16, worse precision.
MM_DTYPE = os.environ.get("DAYMLP_MM_DTYPE", "float16")

_cache: dict = {}
last_run_result = None  # stash of BassKernelResults for test harness use


def _build(mm_dtype_name: str, apply_affine: bool) -> bass.Bass:
    f32 = mybir.dt.float32
    # DRAM inputs and the SBUF tiles feeding the PE carry the matmul dtype
    # directly (for float32r the producing writes perform the required
    # rounding; fp16/bf16 arrays are cast host-side).
    store_dt = getattr(mybir.dt, mm_dtype_name)
    dram_dt = store_dt

    # Bacc (not raw Bass): its compile pipeline moves extra matmul waits onto
    # ldweights and splits >1-wait instructions via event semaphores, which
    # the TRN2 ISA requires.
    nc = bacc.Bacc("TRN2", target_bir_lowering=False)
    # Partition-major DRAM layouts: each SBUF partition's data is one
    # contiguous DRAM run, so every load is 128 large descriptors instead of
    # 128*K small ones (the DMA engines are descriptor-rate limited).
    xt_d = nc.dram_tensor("xt", [S, P, KD, T], dram_dt, kind="ExternalInput")
    w1_d = nc.dram_tensor("w1", [S, P, 2, KD, D_HID // 2], dram_dt, kind="ExternalInput")
    b1_d = nc.dram_tensor("b1", [S, P, KH], f32, kind="ExternalInput")
    w2_d = nc.dram_tensor("w2", [S, P, KH, D_OUT], dram_dt, kind="ExternalInput")
    b2_d = nc.dram_tensor("b2", [S, D_OUT], f32, kind="ExternalInput")
    if apply_affine:
        gm_d = nc.dram_tensor("gm", [S, D_OUT], f32, kind="ExternalInput")
        bt_d = nc.dram_tensor("bt", [S, D_OUT], f32, kind="ExternalInput")
    y_d = nc.dram_tensor("y", [S, T, D_OUT], store_dt, kind="ExternalOutput")

    with tile.TileContext(nc) as tc:
        with (
            tc.tile_pool(name="xw", bufs=2) as xw,
            tc.tile_pool(name="hb", bufs=2) as hb,
            tc.tile_pool(name="vec", bufs=2) as vec,
            tc.tile_pool(name="yp", bufs=6) as yp,
            tc.tile_pool(name="st", bufs=8) as st,
            tc.tile_pool(name="consts", bufs=1) as cpool,
            tc.tile_pool(name="prologue", bufs=1) as pro,
            tc.tile_pool(name="psum", bufs=8, space="PSUM") as pp,
        ):
            eps_t = cpool.tile([P, 1], f32)

            # PE pre-warm: matmuls on a zeroed tile while the first real
            # operands are still in flight. The PE clock-gate (HAM) needs
            # ~3.4us of sustained activity to reach 2.4GHz; warming during
            # the DMA head means the real matmuls run at higher rate sooner.
            # The warm operand is a single-partition [1, P] tile (K=1
            # matmul): a 1-partition GPSIMD memset is near-instant, so the
            # PE starts its warm block as soon as GPSIMD exits the NEFF
            # preamble (~5.9us) instead of waiting ~1.5us for a [P, P]
            # memset. Size the block to end at first-data arrival (~7.5us):
            # real matmuls then run throttled-but-useful until HAM fires
            # (427ns each does real work vs a warm matmul's none).
            n_warm = int(os.environ.get("DAYMLP_WARM_MMS", "16"))
            z_t = cpool.tile([1, P], store_dt, name="z_t")
            nc.gpsimd.memset(z_t, 0.0)
            warm_tiles = [
                pp.tile([P, T], f32, tag="ps", name=f"warm_ps_{w}")
                for w in range(min(4, max(1, n_warm)))
            ]
            for w in range(n_warm):
                nc.tensor.matmul(
                    warm_tiles[w % len(warm_tiles)][:, :P],
                    lhsT=z_t,
                    rhs=z_t,
                    start=True,
                    stop=True,
                )

            HH = D_HID // 2
            for s in range(S):
                if s == 0:
                    # prologue: few, k-granular DMAs, with the DIRECT2D
                    # descriptor generation (~0.65us per DMA regardless of
                    # size) split across the TWO hwdge-capable sequencers
                    # (SP and Activation; DVE can't issue DMAs). On one
                    # sequencer the 8 head DMAs serialize to ~5.2us of
                    # issue, which is exactly what gated the first real
                    # matmul at ~11.6us; split, the k=0 pair (xt0+w10) is
                    # generated by ~7.3us and lands ~8.3us, and chunk k
                    # always lands well before the matmuls consume k-1.
                    xt_ck, w1_cks = [], []
                    for k in range(KD):
                        xk = pro.tile([P, T], store_dt, tag=f"xt0_{k}", name=f"xt0_{k}")
                        nc.sync.dma_start(out=xk, in_=xt_d[s, :, k, :])
                        wk = pro.tile([P, 2, HH], store_dt, tag=f"w10_{k}", name=f"w10_{k}")
                        nc.scalar.dma_start(out=wk, in_=w1_d[s, :, :, k, :])
                        xt_ck.append(xk)
                        w1_cks.append(wk)
                    b1_t = vec.tile([P, KH], f32, tag="b1")
                    nc.sync.dma_start(out=b1_t, in_=b1_d[s])
                    nc.vector.memset(eps_t, EPS)

                    def w1_sl(k, h):
                        half, hh = h // (KH // 2), h % (KH // 2)
                        return w1_cks[k][:, half, P * hh : P * (hh + 1)]
                else:
                    b1_t = vec.tile([P, KH], f32, tag="b1")
                    nc.sync.dma_start(out=b1_t, in_=b1_d[s])
                    xt_t = xw.tile([P, KD, T], store_dt, tag="xt")
                    nc.sync.dma_start(out=xt_t, in_=xt_d[s])
                    # w1 in h-halves: pass 1 h0-3 only gates on the first
                    # half, smoothing the early-delivery cliff
                    w1a_t = xw.tile([P, KD, HH], store_dt, tag="w1a")
                    nc.sync.dma_start(out=w1a_t, in_=w1_d[s, :, 0])
                    w1b_t = xw.tile([P, KD, HH], store_dt, tag="w1b")
                    nc.sync.dma_start(out=w1b_t, in_=w1_d[s, :, 1])

                def load_rest(s=s):
                    if s == 0:
                        qs = []
                        for q in range(4):
                            w2q = xw.tile([P, 2, D_OUT], store_dt, tag=f"w2q{q}")
                            nc.sync.dma_start(out=w2q, in_=w2_d[s, :, 2 * q : 2 * q + 2])
                            qs.append(w2q)
                        w2_t = tuple(qs)
                    else:
                        w2_t = xw.tile([P, KH, D_OUT], store_dt, tag="w2")
                        nc.sync.dma_start(out=w2_t, in_=w2_d[s])
                    b2_t = vec.tile([P, 1, D_OUT], f32, tag="b2")
                    nc.sync.dma_start(
                        out=b2_t, in_=b2_d[s : s + 1, :].partition_broadcast(P)
                    )
                    gm_t = bt_t = None
                    if apply_affine:
                        gm_t = vec.tile([P, 1, D_OUT], f32, tag="gm")
                        nc.sync.dma_start(
                            out=gm_t, in_=gm_d[s : s + 1, :].partition_broadcast(P)
                        )
                        bt_t = vec.tile([P, 1, D_OUT], f32, tag="bt")
                        nc.sync.dma_start(
                            out=bt_t, in_=bt_d[s : s + 1, :].partition_broadcast(P)
                        )
                    return w2_t, b2_t, gm_t, bt_t

                if s > 0:
                    # pass-2 operands up front so DMA overlaps pass-1 compute
                    w2_t, b2_t, gm_t, bt_t = load_rest()

                # pass 1: hT[h, :] = relu(W1[:, h].T @ xT + b1[h])
                hT_t = hb.tile([P, KH, T], store_dt, tag="hT")
                if s == 0:
                    # k-outer over all 8 PSUM banks: matmuls start as soon as
                    # chunk k=0 has landed
                    ps_list = [pp.tile([P, T], f32, tag="ps", name=f"ps0_{h}") for h in range(KH)]
                    for k in range(KD):
                        for h in range(KH):
                            nc.tensor.matmul(
                                ps_list[h],
                                lhsT=w1_sl(k, h),
                                rhs=xt_ck[k],
                                start=(k == 0),
                                stop=(k == KD - 1),
                            )
                    w2_t, b2_t, gm_t, bt_t = load_rest()
                    for h in range(KH):
                        nc.scalar.activation(
                            out=hT_t[:, h, :],
                            in_=ps_list[h],
                            func=mybir.ActivationFunctionType.Relu,
                            bias=b1_t[:, h : h + 1],
                            scale=1.0,
                        )
                else:
                    for h in range(KH):
                        w1h_t = w1a_t if h < KH // 2 else w1b_t
                        hh = h % (KH // 2)
                        ps = pp.tile([P, T], f32, tag="ps")
                        for k in range(KD):
                            nc.tensor.matmul(
                                ps,
                                lhsT=w1h_t[:, k, P * hh : P * (hh + 1)],
                                rhs=xt_t[:, k, :],
                                start=(k == 0),
                                stop=(k == KD - 1),
                            )
                        nc.scalar.activation(
                            out=hT_t[:, h, :],
                            in_=ps,
                            func=mybir.ActivationFunctionType.Relu,
                            bias=b1_t[:, h : h + 1],
                            scale=1.0,
                        )

                # pass 2: y[t_tile, :] = hT[:, t_tile].T @ W2 (+ b2), then LN.
                # Mid-stream samples keep the whole LN chain on the DVE
                # (bias-add, bn_stats/aggr, reciprocal, apply) with only the
                # rsqrt's Sqrt on ACT — DVE is ~54% busy so nothing backs
                # up. The LAST sample's chains are what the kernel drains
                # AFTER the final matmul, and there the DVE per-tile chain
                # (~1.84us) exceeds the 8-matmul tile period (~1.73us), so
                # a backlog forms. For s==S-1 the (y-mean)*rstd apply moves
                # to ACT (Identity(in*scale+bias) with per-partition
                # scale=rstd, bias=-mean*rstd), and the final t-tile is
                # computed in two 256-col PSUM halves so its bias-add and
                # bn_stats overlap the second half's matmuls. Output stays
                # fp16 (host upcasts; LN output is O(1) so fp16 rounding
                # ~5e-4 abs, well under the 2e-2 gate).
                last_s = s == S - 1
                for t in range(MT):
                    split = last_s and t == MT - 1
                    y_t = yp.tile([P, D_OUT], store_dt, tag="y")
                    if split:
                        HO = D_OUT // 2
                        stats = st.tile([P, 2, 6], f32, tag="st2")
                        for i in range(2):
                            # full-bank tile, half used: guarantees the two
                            # halves land in different PSUM banks so the
                            # DVE read of half 0 overlaps PE writes of half 1
                            psh_full = pp.tile([P, D_OUT], f32, tag="ps")
                            psh = psh_full[:, 0:HO]
                            for k in range(KH):
                                w2_rhs = (
                                    w2_t[k // 2][:, k % 2, i * HO : (i + 1) * HO]
                                    if isinstance(w2_t, tuple)
                                    else w2_t[:, k, i * HO : (i + 1) * HO]
                                )
                                nc.tensor.matmul(
                                    psh,
                                    lhsT=hT_t[:, k, P * t : P * (t + 1)],
                                    rhs=w2_rhs,
                                    start=(k == 0),
                                    stop=(k == KH - 1),
                                )
                            nc.vector.tensor_add(
                                out=y_t[:, i * HO : (i + 1) * HO],
                                in0=psh,
                                in1=b2_t[:, 0, i * HO : (i + 1) * HO],
                            )
                            nc.vector.bn_stats(
                                out=stats[:, i, :], in_=y_t[:, i * HO : (i + 1) * HO]
                            )
                    else:
                        ps2 = pp.tile([P, D_OUT], f32, tag="ps")
                        for k in range(KH):
                            w2_rhs = (
                                w2_t[k // 2][:, k % 2, :]
                                if isinstance(w2_t, tuple)
                                else w2_t[:, k, :]
                            )
                            nc.tensor.matmul(
                                ps2,
                                lhsT=hT_t[:, k, P * t : P * (t + 1)],
                                rhs=w2_rhs,
                                start=(k == 0),
                                stop=(k == KH - 1),
                            )
                        # y held in fp16: bn_stats and the LN apply then run
                        # at the DVE's 2x 16-bit rate; fp16 rounding of
                        # pre-LN y is ~5e-4 relative, far under the gate
                        nc.vector.tensor_add(out=y_t, in0=ps2, in1=b2_t[:, 0, :])
                        stats = st.tile([P, 6], f32, tag="stats")
                        nc.vector.bn_stats(out=stats, in_=y_t)
                    mv = st.tile([P, 2], f32, tag="mv")
                    nc.vector.bn_aggr(out=mv, in_=stats)
                    rstd = st.tile([P, 1], f32, tag="rstd")
                    nc.scalar.activation(
                        out=rstd,
                        in_=mv[:, 1:2],
                        func=mybir.ActivationFunctionType.Sqrt,
                        bias=eps_t,
                        scale=1.0,
                    )
                    nc.vector.reciprocal(out=rstd, in_=rstd)
                    y16 = yp.tile([P, D_OUT], store_dt, tag="y16")
                    if last_s and not split:
                        # nm = -mean * rstd; apply on ACT frees the DVE so
                        # the final tile's chain isn't queued behind these
                        nm = st.tile([P, 1], f32, tag="nm")
                        nc.vector.tensor_scalar(
                            out=nm,
                            in0=mv[:, 0:1],
                            scalar1=rstd,
                            scalar2=-1.0,
                            op0=mybir.AluOpType.mult,
                            op1=mybir.AluOpType.mult,
                        )
                        target = y16 if not apply_affine else yp.tile(
                            [P, D_OUT], f32, tag="ya"
                        )
                        nc.scalar.activation(
                            out=target,
                            in_=y_t,
                            func=mybir.ActivationFunctionType.Identity,
                            bias=nm,
                            scale=rstd,
                        )
                        if apply_affine:
                            nc.vector.tensor_mul(out=target, in0=target, in1=gm_t[:, 0, :])
                            nc.vector.tensor_add(out=y16, in0=target, in1=bt_t[:, 0, :])
                    elif apply_affine:
                        ya = yp.tile([P, D_OUT], f32, tag="ya")
                        nc.vector.tensor_scalar(
                            out=ya,
                            in0=y_t,
                            scalar1=mv[:, 0:1],
                            scalar2=rstd,
                            op0=mybir.AluOpType.subtract,
                            op1=mybir.AluOpType.mult,
                        )
                        nc.vector.tensor_mul(out=ya, in0=ya, in1=gm_t[:, 0, :])
                        nc.vector.tensor_add(out=y16, in0=ya, in1=bt_t[:, 0, :])
                    else:
                        nc.vector.tensor_scalar(
                            out=y16,
                            in0=y_t,
                            scalar1=mv[:, 0:1],
                            scalar2=rstd,
                            op0=mybir.AluOpType.subtract,
                            op1=mybir.AluOpType.mult,
                        )
                    if split:
                        # final output DMA issued from the ACT queue: no
                        # cross-engine hop behind the sync sequencer's
                        # earlier DIRECT2D issues at the very tail
                        nc.scalar.dma_start(
                            out=y_d[s, P * t : P * (t + 1), :], in_=y16
                        )
                    else:
                        nc.sync.dma_start(out=y_d[s, P * t : P * (t + 1), :], in_=y16)
    nc.finalize()
    return nc


def kernel(**inputs) -> np.ndarray:
    global last_run_result
    x = np.asarray(inputs["x"], dtype=np.float32)
    day = np.asarray(inputs["day_indices"]).astype(np.int64)
    W1 = np.asarray(inputs["W1"], dtype=np.float32)
    b1 = np.asarray(inputs["b1"], dtype=np.float32)
    W2 = np.asarray(inputs["W2"], dtype=np.float32)
    b2 = np.asarray(inputs["b2"], dtype=np.float32)
    gamma = np.asarray(inputs["gamma"], dtype=np.float32)
    beta = np.asarray(inputs["beta"], dtype=np.float32)

    apply_affine = not (np.all(gamma == 1.0) and np.all(beta == 0.0))
    key = (MM_DTYPE, apply_affine)
    if key not in _cache:
        _cache[key] = _build(*key)
    nc = _cache[key]

    mm_np = {
        "bfloat16": ml_dtypes.bfloat16,
        "float16": np.float16,
    }.get(MM_DTYPE, np.float32)

    # host-side routing gather + layout prep: K on partitions, and
    # partition-major so each partition's DMA data is contiguous in DRAM
    xt = np.ascontiguousarray(
        x.transpose(0, 2, 1).reshape(B, KD, P, T).transpose(0, 2, 1, 3).astype(mm_np)
    )
    # [B, P, half, KD, D_HID//2]: half-major so pass-1 h0-3 gates on one DMA
    W1d = np.ascontiguousarray(
        W1[day]
        .reshape(B, KD, P, 2, D_HID // 2)
        .transpose(0, 2, 3, 1, 4)
        .astype(mm_np)
    )
    W2d = np.ascontiguousarray(
        W2[day].reshape(B, KH, P, D_OUT).transpose(0, 2, 1, 3).astype(mm_np)
    )
    b1d = np.ascontiguousarray(b1[day].reshape(B, KH, P).transpose(0, 2, 1))
    b2d = np.ascontiguousarray(b2[day])
    gmd = np.ascontiguousarray(gamma[day])
    btd = np.ascontiguousarray(beta[day])

    in_maps = []
    for c in range(N_CORES):
        sl = slice(c * S, (c + 1) * S)
        m = {
            "xt": xt[sl],
            "w1": W1d[sl],
            "b1": b1d[sl],
            "w2": W2d[sl],
            "b2": b2d[sl],
        }
        if apply_affine:
            m["gm"] = gmd[sl]
            m["bt"] = btd[sl]
        in_maps.append(m)

    trace = os.environ.get("DAYMLP_TRACE", "0") == "1"
    res = run_bass_kernel_spmd(
        nc,
        in_maps,
        core_ids=list(range(N_CORES)),
        trace=trace,
    )
    last_run_result = res
    y = np.concatenate([np.asarray(r["y"], dtype=np.float32) for r in res.results], axis=0)
    return y



# revision 13
# speedup vs baseline: 1.0056x; 1.0056x over previous
"""Day-routed adapter MLP (per-sample day-specific 2-layer MLP + LayerNorm)
for 8 Trainium2 NeuronCores.

Computation per sample b (day d = day_indices[b]):
    h = relu(x[b] @ W1[d] + b1[d])        # [T, D_hid]
    y = h @ W2[d] + b2[d]                 # [T, D_out]
    out = LN(y) * gamma[d] + beta[d]      # LN over last dim

Sharding: data-parallel over batch, 8 samples per core. The per-sample
day weights are gathered on the host (routing is host-visible), and x is
pre-transposed on the host so the device needs no transposes at all:

  pass 1:  hT[h_chunk, :T] += W1[k_chunk, h_chunk].T @ xT[k_chunk, :T]
           (lhsT = W1 natural layout, rhs = xT)  -> hT with H on partitions,
           so b1 is a per-partition bias fused into the ReLU copyback (ACT).
  pass 2:  y[t_tile, :O]  += hT[k_chunk, t_tile].T @ W2[k_chunk, :O]
           (lhsT = hT from pass 1, rhs = W2 natural layout) -> y with T on
           partitions and O on the free axis, which is exactly the layout
           LayerNorm wants (bn_stats/bn_aggr reduce along free axis).
"""

import os

import numpy as np
import ml_dtypes

import concourse.bass as bass
import concourse.mybir as mybir
import concourse.tile as tile
from concourse import bacc
from concourse.bass_utils import run_bass_kernel_spmd

N_CORES = 8
B, T, D_IN = 64, 512, 512
D_HID, D_OUT = 1024, 512
S = B // N_CORES  # samples per core
EPS = 1e-5

P = 128
KD = D_IN // P   # 4 contraction chunks in pass 1
KH = D_HID // P  # 8 contraction chunks in pass 2 (= H chunks of pass 1 out)
MT = T // P      # 4 token tiles in pass 2

# Matmul input dtype. float16: full PE rate (1 cyc/row, FWL hides weight
# loads), half the DMA bytes of fp32, and a 10-bit mantissa (~4x better than
# bf16; fp32 accumulate in PSUM). float32r: fp32 storage but ~2 cyc/row and
# 2x the DMA traffic. bfloat16: same speed as fp16, worse precision.
MM_DTYPE = os.environ.get("DAYMLP_MM_DTYPE", "float16")

_cache: dict = {}
last_run_result = None  # stash of BassKernelResults for test harness use


def _build(mm_dtype_name: str, apply_affine: bool) -> bass.Bass:
    f32 = mybir.dt.float32
    # DRAM inputs and the SBUF tiles feeding the PE carry the matmul dtype
    # directly (for float32r the producing writes perform the required
    # rounding; fp16/bf16 arrays are cast host-side).
    store_dt = getattr(mybir.dt, mm_dtype_name)
    dram_dt = store_dt

    # Bacc (not raw Bass): its compile pipeline moves extra matmul waits onto
    # ldweights and splits >1-wait instructions via event semaphores, which
    # the TRN2 ISA requires.
    nc = bacc.Bacc("TRN2", target_bir_lowering=False)
    # Partition-major DRAM layouts: each SBUF partition's data is one
    # contiguous DRAM run, so every load is 128 large descriptors instead of
    # 128*K small ones (the DMA engines are descriptor-rate limited).
    xt_d = nc.dram_tensor("xt", [S, P, KD, T], dram_dt, kind="ExternalInput")
    w1_d = nc.dram_tensor("w1", [S, P, 2, KD, D_HID // 2], dram_dt, kind="ExternalInput")
    b1_d = nc.dram_tensor("b1", [S, P, KH], f32, kind="ExternalInput")
    w2_d = nc.dram_tensor("w2", [S, P, KH, D_OUT], dram_dt, kind="ExternalInput")
    b2_d = nc.dram_tensor("b2", [S, D_OUT], f32, kind="ExternalInput")
    if apply_affine:
        gm_d = nc.dram_tensor("gm", [S, D_OUT], f32, kind="ExternalInput")
        bt_d = nc.dram_tensor("bt", [S, D_OUT], f32, kind="ExternalInput")
    y_d = nc.dram_tensor("y", [S, T, D_OUT], store_dt, kind="ExternalOutput")

    with tile.TileContext(nc) as tc:
        with (
            tc.tile_pool(name="xw", bufs=2) as xw,
            tc.tile_pool(name="hb", bufs=2) as hb,
            tc.tile_pool(name="vec", bufs=2) as vec,
            tc.tile_pool(name="yp", bufs=6) as yp,
            tc.tile_pool(name="st", bufs=8) as st,
            tc.tile_pool(name="consts", bufs=1) as cpool,
            tc.tile_pool(name="prologue", bufs=1) as pro,
            tc.tile_pool(name="psum", bufs=8, space="PSUM") as pp,
        ):
            eps_t = cpool.tile([P, 1], f32)

            # PE pre-warm: matmuls on a zeroed tile while the first real
            # operands are still in flight. The PE clock-gate (HAM) needs
            # ~3.4us of sustained activity to reach 2.4GHz; warming during
            # the DMA head means the real matmuls run at higher rate sooner.
            # The warm operand is a single-partition [1, P] tile (K=1
            # matmul): a 1-partition GPSIMD memset is near-instant, so the
            # PE starts its warm block as soon as GPSIMD exits the NEFF
            # preamble (~5.9us) instead of waiting ~1.5us for a [P, P]
            # memset. Size the block to end at first-data arrival (~7.5us):
            # real matmuls then run throttled-but-useful until HAM fires
            # (427ns each does real work vs a warm matmul's none).
            n_warm = int(os.environ.get("DAYMLP_WARM_MMS", "30"))
            z_t = cpool.tile([1, P], store_dt, name="z_t")
            nc.gpsimd.memset(z_t, 0.0)
            warm_tiles = [
                pp.tile([P, T], f32, tag="ps", name=f"warm_ps_{w}")
                for w in range(min(4, max(1, n_warm)))
            ]
            for w in range(n_warm):
                nc.tensor.matmul(
                    warm_tiles[w % len(warm_tiles)][:, :P],
                    lhsT=z_t,
                    rhs=z_t,
                    start=True,
                    stop=True,
                )

            HH = D_HID // 2
            for s in range(S):
                if s == 0:
                    # prologue head economics (all measured): DIRECT2D
                    # descriptor generation costs ~0.65us per DMA on the
                    # issuing sequencer REGARDLESS of size; the queues only
                    # start draining ~0.75us after the FIRST gen ends, then
                    # process ~100ns per descriptor per queue FIFO. So the
                    # first matmul's gate is (queue-arm time) + (FIFO
                    # position of its last descriptor). Priority-order
                    # everything on ONE sequencer (a second engine's
                    # descriptors interleave into the same queue FIFOs
                    # ahead of later critical pieces — measured +1.7us
                    # regression). xt is ONE coarse DMA (128 descriptors of
                    # 4KB — cheap positions, covers all k), and w1 is split
                    # into k-granular h-halves (128x1KB each) so the first
                    # 4 matmuls gate on only xt + one 128KB piece.
                    xt_t = xw.tile([P, KD, T], store_dt, tag="xt")
                    nc.sync.dma_start(out=xt_t, in_=xt_d[s])
                    w1p = {}
                    for k in range(KD):
                        for half in range(2):
                            wp = pro.tile(
                                [P, HH], store_dt,
                                tag=f"w1_{k}_{half}", name=f"w1_{k}_{half}",
                            )
                            nc.sync.dma_start(out=wp, in_=w1_d[s, :, half, k, :])
                            w1p[(k, half)] = wp
                    b1_t = vec.tile([P, KH], f32, tag="b1")
                    nc.sync.dma_start(out=b1_t, in_=b1_d[s])
                    nc.vector.memset(eps_t, EPS)

                    def w1_sl(k, h):
                        half, hh = h // (KH // 2), h % (KH // 2)
                        return w1p[(k, half)][:, P * hh : P * (hh + 1)]
                else:
                    b1_t = vec.tile([P, KH], f32, tag="b1")
                    nc.sync.dma_start(out=b1_t, in_=b1_d[s])
                    xt_t = xw.tile([P, KD, T], store_dt, tag="xt")
                    nc.sync.dma_start(out=xt_t, in_=xt_d[s])
                    # w1 in h-halves: pass 1 h0-3 only gates on the first
                    # half, smoothing the early-delivery cliff
                    w1a_t = xw.tile([P, KD, HH], store_dt, tag="w1a")
                    nc.sync.dma_start(out=w1a_t, in_=w1_d[s, :, 0])
                    w1b_t = xw.tile([P, KD, HH], store_dt, tag="w1b")
                    nc.sync.dma_start(out=w1b_t, in_=w1_d[s, :, 1])

                def load_rest(s=s):
                    if s == 0:
                        qs = []
                        for q in range(4):
                            w2q = xw.tile([P, 2, D_OUT], store_dt, tag=f"w2q{q}")
                            nc.sync.dma_start(out=w2q, in_=w2_d[s, :, 2 * q : 2 * q + 2])
                            qs.append(w2q)
                        w2_t = tuple(qs)
                    else:
                        w2_t = xw.tile([P, KH, D_OUT], store_dt, tag="w2")
                        nc.sync.dma_start(out=w2_t, in_=w2_d[s])
                    b2_t = vec.tile([P, 1, D_OUT], f32, tag="b2")
                    nc.sync.dma_start(
                        out=b2_t, in_=b2_d[s : s + 1, :].partition_broadcast(P)
                    )
                    gm_t = bt_t = None
                    if apply_affine:
                        gm_t = vec.tile([P, 1, D_OUT], f32, tag="gm")
                        nc.sync.dma_start(
                            out=gm_t, in_=gm_d[s : s + 1, :].partition_broadcast(P)
                        )
                        bt_t = vec.tile([P, 1, D_OUT], f32, tag="bt")
                        nc.sync.dma_start(
                            out=bt_t, in_=bt_d[s : s + 1, :].partition_broadcast(P)
                        )
                    return w2_t, b2_t, gm_t, bt_t

                if s > 0:
                    # pass-2 operands up front so DMA overlaps pass-1 compute
                    w2_t, b2_t, gm_t, bt_t = load_rest()

                # pass 1: hT[h, :] = relu(W1[:, h].T @ xT + b1[h])
                hT_t = hb.tile([P, KH, T], store_dt, tag="hT")
                if s == 0:
                    # k-outer over all 8 PSUM banks: matmuls start as soon as
                    # chunk k=0 has landed
                    ps_list = [pp.tile([P, T], f32, tag="ps", name=f"ps0_{h}") for h in range(KH)]
                    for k in range(KD):
                        for h in range(KH):
                            nc.tensor.matmul(
                                ps_list[h],
                                lhsT=w1_sl(k, h),
                                rhs=xt_t[:, k, :],
                                start=(k == 0),
                                stop=(k == KD - 1),
                            )
                    w2_t, b2_t, gm_t, bt_t = load_rest()
                    for h in range(KH):
                        nc.scalar.activation(
                            out=hT_t[:, h, :],
                            in_=ps_list[h],
                            func=mybir.ActivationFunctionType.Relu,
                            bias=b1_t[:, h : h + 1],
                            scale=1.0,
                        )
                else:
                    for h in range(KH):
                        w1h_t = w1a_t if h < KH // 2 else w1b_t
                        hh = h % (KH // 2)
                        ps = pp.tile([P, T], f32, tag="ps")
                        for k in range(KD):
                            nc.tensor.matmul(
                                ps,
                                lhsT=w1h_t[:, k, P * hh : P * (hh + 1)],
                                rhs=xt_t[:, k, :],
                                start=(k == 0),
                                stop=(k == KD - 1),
                            )
                        nc.scalar.activation(
                            out=hT_t[:, h, :],
                            in_=ps,
                            func=mybir.ActivationFunctionType.Relu,
                            bias=b1_t[:, h : h + 1],
                            scale=1.0,
                        )

                # pass 2: y[t_tile, :] = hT[:, t_tile].T @ W2 (+ b2), then LN.
                # Mid-stream samples keep the whole LN chain on the DVE
                # (bias-add, bn_stats/aggr, reciprocal, apply) with only the
                # rsqrt's Sqrt on ACT — DVE is ~54% busy so nothing backs
                # up. The LAST sample's chains are what the kernel drains
                # AFTER the final matmul, and there the DVE per-tile chain
                # (~1.84us) exceeds the 8-matmul tile period (~1.73us), so
                # a backlog forms. For s==S-1 the (y-mean)*rstd apply moves
                # to ACT (Identity(in*scale+bias) with per-partition
                # scale=rstd, bias=-mean*rstd), and the final t-tile is
                # computed in two 256-col PSUM halves so its bias-add and
                # bn_stats overlap the second half's matmuls. Output stays
                # fp16 (host upcasts; LN output is O(1) so fp16 rounding
                # ~5e-4 abs, well under the 2e-2 gate).
                last_s = s == S - 1
                for t in range(MT):
                    split = last_s and t == MT - 1
                    y_t = yp.tile([P, D_OUT], store_dt, tag="y")
                    if split:
                        HO = D_OUT // 2
                        stats = st.tile([P, 2, 6], f32, tag="st2")
                        for i in range(2):
                            # full-bank tile, half used: guarantees the two
                            # halves land in different PSUM banks so the
                            # DVE read of half 0 overlaps PE writes of half 1
                            psh_full = pp.tile([P, D_OUT], f32, tag="ps")
                            psh = psh_full[:, 0:HO]
                            for k in range(KH):
                                w2_rhs = (
                                    w2_t[k // 2][:, k % 2, i * HO : (i + 1) * HO]
                                    if isinstance(w2_t, tuple)
                                    else w2_t[:, k, i * HO : (i + 1) * HO]
                                )
                                nc.tensor.matmul(
                                    psh,
                                    lhsT=hT_t[:, k, P * t : P * (t + 1)],
                                    rhs=w2_rhs,
                                    start=(k == 0),
                                    stop=(k == KH - 1),
                                )
                            nc.vector.tensor_add(
                                out=y_t[:, i * HO : (i + 1) * HO],
                                in0=psh,
                                in1=b2_t[:, 0, i * HO : (i + 1) * HO],
                            )
                            nc.vector.bn_stats(
                                out=stats[:, i, :], in_=y_t[:, i * HO : (i + 1) * HO]
                            )
                    else:
                        ps2 = pp.tile([P, D_OUT], f32, tag="ps")
                        for k in range(KH):
                            w2_rhs = (
                                w2_t[k // 2][:, k % 2, :]
                                if isinstance(w2_t, tuple)
                                else w2_t[:, k, :]
                            )
                            nc.tensor.matmul(
                                ps2,
                                lhsT=hT_t[:, k, P * t : P * (t + 1)],
                                rhs=w2_rhs,
                                start=(k == 0),
                                stop=(k == KH - 1),
                            )
                        # y held in fp16: bn_stats and the LN apply then run
                        # at the DVE's 2x 16-bit rate; fp16 rounding of
                        # pre-LN y is ~5e-4 relative, far under the gate
                        nc.vector.tensor_add(out=y_t, in0=ps2, in1=b2_t[:, 0, :])
                        stats = st.tile([P, 6], f32, tag="stats")
                        nc.vector.bn_stats(out=stats, in_=y_t)
                    mv = st.tile([P, 2], f32, tag="mv")
                    nc.vector.bn_aggr(out=mv, in_=stats)
                    rstd = st.tile([P, 1], f32, tag="rstd")
                    nc.scalar.activation(
                        out=rstd,
                        in_=mv[:, 1:2],
                        func=mybir.ActivationFunctionType.Sqrt,
                        bias=eps_t,
                        scale=1.0,
                    )
                    nc.vector.reciprocal(out=rstd, in_=rstd)
                    y16 = yp.tile([P, D_OUT], store_dt, tag="y16")
                    if last_s and not split:
                        # nm = -mean * rstd; apply on ACT frees the DVE so
                        # the final tile's chain isn't queued behind these
                        nm = st.tile([P, 1], f32, tag="nm")
                        nc.vector.tensor_scalar(
                            out=nm,
                            in0=mv[:, 0:1],
                            scalar1=rstd,
                            scalar2=-1.0,
                            op0=mybir.AluOpType.mult,
                            op1=mybir.AluOpType.mult,
                        )
                        target = y16 if not apply_affine else yp.tile(
                            [P, D_OUT], f32, tag="ya"
                        )
                        nc.scalar.activation(
                            out=target,
                            in_=y_t,
                            func=mybir.ActivationFunctionType.Identity,
                            bias=nm,
                            scale=rstd,
                        )
                        if apply_affine:
                            nc.vector.tensor_mul(out=target, in0=target, in1=gm_t[:, 0, :])
                            nc.vector.tensor_add(out=y16, in0=target, in1=bt_t[:, 0, :])
                    elif apply_affine:
                        ya = yp.tile([P, D_OUT], f32, tag="ya")
                        nc.vector.tensor_scalar(
                            out=ya,
                            in0=y_t,
                            scalar1=mv[:, 0:1],
                            scalar2=rstd,
                            op0=mybir.AluOpType.subtract,
                            op1=mybir.AluOpType.mult,
                        )
                        nc.vector.tensor_mul(out=ya, in0=ya, in1=gm_t[:, 0, :])
                        nc.vector.tensor_add(out=y16, in0=ya, in1=bt_t[:, 0, :])
                    else:
                        nc.vector.tensor_scalar(
                            out=y16,
                            in0=y_t,
                            scalar1=mv[:, 0:1],
                            scalar2=rstd,
                            op0=mybir.AluOpType.subtract,
                            op1=mybir.AluOpType.mult,
                        )
                    if split:
                        # final output DMA issued from the ACT queue: no
                        # cross-engine hop behind the sync sequencer's
                        # earlier DIRECT2D issues at the very tail
                        nc.scalar.dma_start(
                            out=y_d[s, P * t : P * (t + 1), :], in_=y16
                        )
                    else:
                        nc.sync.dma_start(out=y_d[s, P * t : P * (t + 1), :], in_=y16)
    nc.finalize()
    return nc


def kernel(**inputs) -> np.ndarray:
    global last_run_result
    x = np.asarray(inputs["x"], dtype=np.float32)
    day = np.asarray(inputs["day_indices"]).astype(np.int64)
    W1 = np.asarray(inputs["W1"], dtype=np.float32)
    b1 = np.asarray(inputs["b1"], dtype=np.float32)
    W2 = np.asarray(inputs["W2"], dtype=np.float32)
    b2 = np.asarray(inputs["b2"], dtype=np.float32)
    gamma = np.asarray(inputs["gamma"], dtype=np.float32)
    beta = np.asarray(inputs["beta"], dtype=np.float32)

    apply_affine = not (np.all(gamma == 1.0) and np.all(beta == 0.0))
    key = (MM_DTYPE, apply_affine)
    if key not in _cache:
        _cache[key] = _build(*key)
    nc = _cache[key]

    mm_np = {
        "bfloat16": ml_dtypes.bfloat16,
        "float16": np.float16,
    }.get(MM_DTYPE, np.float32)

    # host-side routing gather + layout prep: K on partitions, and
    # partition-major so each partition's DMA data is contiguous in DRAM
    xt = np.ascontiguousarray(
        x.transpose(0, 2, 1).reshape(B, KD, P, T).transpose(0, 2, 1, 3).astype(mm_np)
    )
    # [B, P, half, KD, D_HID//2]: half-major so pass-1 h0-3 gates on one DMA
    W1d = np.ascontiguousarray(
        W1[day]
        .reshape(B, KD, P, 2, D_HID // 2)
        .transpose(0, 2, 3, 1, 4)
        .astype(mm_np)
    )
    W2d = np.ascontiguousarray(
        W2[day].reshape(B, KH, P, D_OUT).transpose(0, 2, 1, 3).astype(mm_np)
    )
    b1d = np.ascontiguousarray(b1[day].reshape(B, KH, P).transpose(0, 2, 1))
    b2d = np.ascontiguousarray(b2[day])
    gmd = np.ascontiguousarray(gamma[day])
    btd = np.ascontiguousarray(beta[day])

    in_maps = []
    for c in range(N_CORES):
        sl = slice(c * S, (c + 1) * S)
        m = {
            "xt": xt[sl],
            "w1": W1d[sl],
            "b1": b1d[sl],
            "w2": W2d[sl],
            "b2": b2d[sl],
        }
        if apply_affine:
            m["gm"] = gmd[sl]
            m["bt"] = btd[sl]
        in_maps.append(m)

    trace = os.environ.get("DAYMLP_TRACE", "0") == "1"
    res = run_bass_kernel_spmd(
        nc,
        in_maps,
        core_ids=list(range(N_CORES)),
        trace=trace,
    )
    last_run_result = res
    y = np.concatenate([np.asarray(r["y"], dtype=np.float32) for r in res.results], axis=0)
    return y

